# revision 2
# baseline (speedup 1.0000x reference)
# GATConv kernel for Trainium2 (Bass/Tile), 8-core data parallel over batch.
#
# Problem (hardcoded from nn_GATConv_54692113547387):
#   x   [8, 1024, 128] f32, adj [8, 1024, 1024] i32,
#   W   [128, 128] f32,  b [128] f32,  a [64] f32
#   out [8, 1024, 128] f32
#   h = x @ W.T + b, viewed [N, H=4, D=32]
#   e[h,i,j] = leaky_relu(s[h,i] + t[h,j], 0.2); masked where adj==0
#   attn = softmax_j(e);  out[i,(h,d)] = sum_j attn[h,i,j] h[j,h,d]
#
# Math (exact reformulation):
#   exp(lrelu(u)) = max(exp(u), exp(0.2u)) for u = s_i + t_j.  Dividing row i
#   by 32*exp(0.8 s_i) (cancels in softmax):
#     P[j,i] = adj[i,j] * z[j,i],  z = max(sE_i * tE_j, D_j)
#   with sE = exp(0.8 s)/32, tE = exp(t), D = exp(0.2 t)/32 -- all N-sized
#   vectors, so the only N x N work is one 4x-rate dual-op tensor_scalar (z)
#   plus one 2x-rate tensor_tensor (mask multiply) per head/block.
#   out_unnorm^T[(h,d)|sum, i] = sum_j [H_h | 1][j,:]^T P[j,i]  (PE matmul,
#   stationary [33] incl. a ones column giving the softmax denominator),
#   then out[i,hd] = U[d,i]/U[32,i].
#
# Schedule shape (from TimelineSim analysis):
#   - All DMA transfers serialize on one modeled DMA-engine pool, and each
#     adjacency transpose writes an i-column block across every j row, so
#     adjt columns 0:512 land ~11us, full adjt ~16us.
#   - Mask work is therefore split by i-halves and ordered pair01-h0,
#     pair01-h1, pair23; z tiles (no adj dependency) fill DVE until then.
#   - GPSIMD (Pool) takes one head of selected (pair, half, jt) units as a
#     mult single; DVE does the other head at 4x tensor_tensor-free rate...
#     (DVE singles are 327ns [P,512] f16 TT; merged pairs 594ns).
import numpy as np

import concourse.mybir as mybir
import concourse.tile as tile
from concourse import bacc
from concourse.masks import make_identity

F32 = mybir.dt.float32
BF16 = mybir.dt.bfloat16
F16 = mybir.dt.float16
I32 = mybir.dt.int32

P = 128          # partitions
N = 1024         # nodes
NT = N // P      # 8 node tiles
H = 4            # heads
D = 32           # head dim
DE = D + 1       # head dim + rowsum column
NCORES = 8
LN32 = float(np.log(32.0))

# Tuning knobs (module-level so experiments can override before build).
W_BUFS = 6
# (pair, ih, jt) units whose even head runs on Pool as a mult single
# (odd head becomes a DVE single); others are one DVE TT merged over the
# head pair.  pair 0 = heads (0,1), pair 1 = heads (2,3).
POOL_UNITS = {
    (0, 0, 2), (0, 0, 4), (0, 0, 6),
    (0, 1, 1), (0, 1, 3), (0, 1, 5), (0, 1, 7),
    (1, 0, 0), (1, 0, 2), (1, 0, 4), (1, 0, 6),
    (1, 1, 1), (1, 1, 3), (1, 1, 5),
}


def build_nc():
    nc = bacc.Bacc("TRN2", target_bir_lowering=False, debug=False,
                   dynamic_dma_scratch_size=32768)

    x_d = nc.dram_tensor("x", [N, P], F32, kind="ExternalInput")
    adj_d = nc.dram_tensor("adj", [N, N], I32, kind="ExternalInput")
    w_d = nc.dram_tensor("W", [P, P], F32, kind="ExternalInput")
    b_d = nc.dram_tensor("b", [P], F32, kind="ExternalInput")
    a_d = nc.dram_tensor("a", [2 * D], F32, kind="ExternalInput")
    out_d = nc.dram_tensor("out", [N, P], F32, kind="ExternalOutput")

    x_view = x_d[:].rearrange("(t p) i -> p t i", p=P)      # [128, 8, 128]
    adj_view = adj_d[:].rearrange("(t p) j -> p t j", p=P)  # [128, 8, 1024]
    out_view = out_d[:].rearrange("(t p) o -> p t o", p=P)  # [128, 8, 128]

    with tile.TileContext(nc) as tc:
        with (
            tc.tile_pool(name="const", bufs=1) as cpool,
            tc.tile_pool(name="work", bufs=W_BUFS) as wpool,
            tc.tile_pool(name="outp", bufs=3) as opool,
            tc.tile_pool(name="adjfp", bufs=2) as afpool,
            tc.tile_pool(name="psmisc", bufs=2, space="PSUM") as psmisc,
            tc.tile_pool(name="psagg", bufs=4, space="PSUM") as psagg,
            tc.tile_pool(name="psout", bufs=2, space="PSUM") as psout,
        ):
            # -------- ACT table warmup + identity --------
            warm = cpool.tile([1, 8], F32, tag="warm")
            nc.vector.memset(warm[:], 0.0)
            nc.scalar.activation(warm[:], warm[:],
                                 mybir.ActivationFunctionType.Exp)
            ident = cpool.tile([P, P], F32, tag="ident")
            make_identity(nc, ident[:])

            # -------- input DMAs (shared serial HWDGE: order matters) ------
            x_sb = cpool.tile([P, NT, P], F32, tag="x")
            for g in range(2):
                nc.sync.dma_start(x_sb[:, g * 4:(g + 1) * 4, :],
                                  x_view[:, g * 4:(g + 1) * 4, :])
            # a2[d, c]: c=0 -> a_src[d], c=1 -> a_dst[d]
            a2 = cpool.tile([D, 2], F32, tag="a2")
            nc.sync.dma_start(a2[:], a_d[:].rearrange("(c d) -> d c", c=2))
            w_sb = cpool.tile([P, P], F32, tag="w")
            nc.sync.dma_start(w_sb[:], w_d[:])
            bias_col = cpool.tile([P, 1], F32, tag="bias")
            nc.sync.dma_start(bias_col[:], b_d[:, None])

            # -------- adjacency pipeline (the long pole) --------
            # SWDGE cast-load (int32->f16) on Pool into rotating staging
            # tiles (a single tile would create false WAR serialization),
            # each chased by its xbar transpose.
            adjt = cpool.tile([P, NT, N], F16, tag="adjt")
            for g in range(2):
                adj_f = afpool.tile([P, 4, N], F16, tag="adjf",
                                    name=f"adjf{g}")
                nc.gpsimd.dma_start(adj_f[:],
                                    adj_view[:, 4 * g:4 * g + 4, :])
                for q in range(4):
                    it = 4 * g + q
                    nc.sync.dma_start_transpose(
                        adjt[:, :, it * P:(it + 1) * P], adj_f[:, q, :]
                    )

            # -------- small on-chip constants --------
            # ident4[k, p] = (k == p % 32): replicates a2 into all 4 blocks
            ident4 = cpool.tile([D, P], F32, tag="ident4")
            for h in range(H):
                nc.vector.tensor_copy(ident4[:, h * D:(h + 1) * D],
                                      ident[0:D, 0:D])
            # one-hot broadcast stationaries oh[c, h*128+p] = (c == h)
            ohT = cpool.tile([P, H * H], F32, tag="ohT")
            nc.vector.memset(ohT[:], 0.0)
            for h in range(H):
                nc.vector.memset(ohT[:, h * H + h:h * H + h + 1], 1.0)
            oh = cpool.tile([H, H * P], F16, tag="oh")
            mln32 = cpool.tile([P, 1], F32, tag="mln32")
            nc.vector.memset(mln32[:], -LN32)

            # persistent tiles
            xt_sb = cpool.tile([P, N], BF16, tag="xt")
            wt_sb = cpool.tile([P, P], BF16, tag="wt")
            v8_sb = cpool.tile([P, 2 * H], BF16, tag="v8")
            cs_col = cpool.tile([H, 1], F32, tag="cs")
            ct_col = cpool.tile([H, 1], F32, tag="ct")
            c08 = cpool.tile([H, 1], F32, tag="c08")
            s16e = cpool.tile([H, N], F16, tag="s16e")
            t_sb = cpool.tile([H, N], F32, tag="t")
            sbc = cpool.tile([P, H, N], F16, tag="sbc")
            dcols = cpool.tile([P, NT, H], F32, tag="dcols")
            ecols = cpool.tile([P, NT, H], F32, tag="ecols")
            ht_sb = cpool.tile([P, N], BF16, tag="ht")
            hext = cpool.tile([P, NT, H * DE], F16, tag="hext")
            outT = [
                cpool.tile([DE, N], F32, tag=f"outT{h}", name=f"outT{h}")
                for h in range(H)
            ]
            out_sb = cpool.tile([P, NT, P], F32, tag="outsb")
            ident16 = cpool.tile([P, P], BF16, tag="ident16")

            # ---------------- features ----------------
            # oh via base-0 memsets + PE transposes (partition-base rules
            # forbid writing row h directly)
            ps = psmisc.tile([P, 512], F32, tag="m")
            for h in range(H):
                nc.tensor.transpose(ps[0:H, h * P:(h + 1) * P],
                                    ohT[:, h * H:(h + 1) * H], ident[:])
            nc.scalar.copy(oh[:], ps[0:H, :])

            # a_rep = a2 replicated to all head blocks (PE), then ab_bd via
            # partition-sliced DVE copies (no partition shifts)
            ps = psmisc.tile([P, 512], F32, tag="m")
            nc.tensor.matmul(ps[:, 0:2], ident4[:], a2[:],
                             start=True, stop=True)
            a_rep = cpool.tile([P, 2], F32, tag="arep")
            nc.vector.tensor_copy(a_rep[:], ps[:, 0:2])
            ab_bd = cpool.tile([P, 2 * H], F32, tag="ab")
            nc.vector.memset(ab_bd[:], 0.0)
            for h in range(H):
                sl = slice(h * D, (h + 1) * D)
                for j in range(2):
                    nc.vector.tensor_copy(ab_bd[sl, j * H + h:j * H + h + 1],
                                          a_rep[sl, j:j + 1])

            # V8[i, c] = sum_o W[o, i] * ab_bd[o, c]; c-consts via bias row
            ps = psmisc.tile([P, 512], F32, tag="m")
            nc.tensor.matmul(ps[:, 0:2 * H], w_sb[:], ab_bd[:],
                             start=True, stop=True)
            nc.vector.tensor_copy(v8_sb[:], ps[:, 0:2 * H])
            ps = psmisc.tile([P, 512], F32, tag="m")
            nc.tensor.matmul(ps[0:1, 0:2 * H], bias_col[:], ab_bd[:],
                             start=True, stop=True)
            c_row = cpool.tile([1, 2 * H], F32, tag="crow")
            nc.vector.tensor_copy(c_row[:], ps[0:1, 0:2 * H])
            ps2 = psmisc.tile([P, 512], F32, tag="m")
            nc.tensor.transpose(ps2[0:H, 0:1], c_row[:, 0:H],
                                ident[0:1, 0:1])
            nc.tensor.transpose(ps2[0:H, 1:2], c_row[:, H:2 * H],
                                ident[0:1, 0:1])
            nc.vector.tensor_copy(cs_col[:], ps2[0:H, 0:1])
            nc.vector.tensor_copy(ct_col[:], ps2[0:H, 1:2])
            # c08 = 0.8*c_s - ln(32)  (the /32 softmax rescale rides the exp)
            nc.vector.tensor_scalar(c08[:], cs_col[:], 0.8, -LN32,
                                    mybir.AluOpType.mult,
                                    mybir.AluOpType.add)

            # xT[i, n] = x[n, i]: 4 transposes per PSUM bank, 1 ACT copy
            # (bf16: feeds 1-cycle/row matmuls; feature-path precision is
            # ~3e-3 relative, well under the 2e-2 gate)
            for g in range(2):
                ps = psmisc.tile([P, 512], F32, tag="m")
                for k in range(4):
                    t = g * 4 + k
                    nc.tensor.transpose(ps[:, k * P:(k + 1) * P],
                                        x_sb[:, t, :], ident[:])
                nc.scalar.copy(xt_sb[:, g * 512:(g + 1) * 512], ps[:])

            # s/t rows [4, N] straight from xT (W folded into a via V8):
            # sE = exp(0.8 s + 0.8 c_s - ln32), t = raw + c_t
            for half in range(2):
                sl = slice(half * 512, (half + 1) * 512)
                ps = psmisc.tile([P, 512], F32, tag="m")
                nc.tensor.matmul(ps[0:H, :], v8_sb[:, 0:H], xt_sb[:, sl],
                                 start=True, stop=True)
                nc.scalar.activation(
                    s16e[:, sl], ps[0:H, :],
                    mybir.ActivationFunctionType.Exp,
                    bias=c08[:], scale=0.8,
                )
                ps2 = psmisc.tile([P, 512], F32, tag="m")
                nc.tensor.matmul(ps2[0:H, :], v8_sb[:, H:2 * H],
                                 xt_sb[:, sl], start=True, stop=True)
                nc.scalar.add(t_sb[:, sl], ps2[0:H, :], ct_col[:])

            # tT then D/tE columns (before the sbc copies on in-order ACT:
            # the first z needs them)
            for g in range(2):
                ps = psmisc.tile([P, 512], F32, tag="m")
                for k in range(4):
                    t = g * 4 + k
                    nc.tensor.transpose(
                        ps[:, k * H:(k + 1) * H],
                        t_sb[:, t * P:(t + 1) * P], ident[0:H, 0:H]
                    )
                psv = ps[:, 0:4 * H].rearrange("p (t h) -> p t h", h=H)
                nc.scalar.activation(
                    dcols[:, g * 4:(g + 1) * 4, :], psv,
                    mybir.ActivationFunctionType.Exp,
                    bias=mln32[:], scale=0.2,
                )
                nc.scalar.activation(
                    ecols[:, g * 4:(g + 1) * 4, :], psv,
                    mybir.ActivationFunctionType.Exp,
                )

            # sbc[p, h, n] = sE[h, n] on all partitions, via PE one-hot
            # broadcast matmuls (no DRAM bounce)
            for h in range(H):
                for half in range(2):
                    sl = slice(half * 512, (half + 1) * 512)
                    ps = psmisc.tile([P, 512], F32, tag="m")
                    nc.tensor.matmul(ps[:], oh[:, h * P:(h + 1) * P],
                                     s16e[:, sl], start=True, stop=True)
                    nc.scalar.copy(sbc[:, h, sl], ps[:])

            # WT; hT[o, n] = WT^T xT + b; hext = hT^T + ones column
            ps = psmisc.tile([P, 512], F32, tag="m")
            nc.tensor.transpose(ps[:, 0:P], w_sb[:], ident[:])
            nc.scalar.copy(wt_sb[:], ps[:, 0:P])
            nc.scalar.copy(ident16[:], ident[:])
            for half in range(2):
                sl = slice(half * 512, (half + 1) * 512)
                ps = psmisc.tile([P, 512], F32, tag="m")
                nc.tensor.matmul(ps[:], wt_sb[:], xt_sb[:, sl],
                                 start=True, stop=True)
                nc.scalar.add(ht_sb[:, sl], ps[:], bias_col[:])
            for g in range(2):
                ps = psmisc.tile([P, 512], F32, tag="m")
                psb = ps[:].bitcast(BF16)  # [P, 1024] bf16 view
                for k in range(4):
                    t = g * 4 + k
                    nc.tensor.transpose(psb[:, k * P:(k + 1) * P],
                                        ht_sb[:, t * P:(t + 1) * P],
                                        ident16[:])
                dst = (hext[:, g * 4:(g + 1) * 4, :]
                       .rearrange("p t (h e) -> p t h e", h=H)[:, :, :, 0:D])
                srcap = psb[:, 0:512].rearrange("p (t h e) -> p t h e",
                                                t=4, h=H)
                nc.scalar.copy(dst, srcap)
            ones_ap = hext[:].rearrange("p t (h e) -> p t h e", h=H)[:, :, :, D]
            nc.vector.memset(ones_ap, 1.0)

            # ---------------- main loop ----------------
            def z_op(dst, h, jt):
                # z = max(sE_i * tE_j, D_j): one 4x dual-op tensor_scalar
                nc.vector.tensor_scalar(
                    dst, sbc[:, h, :],
                    ecols[:, jt, h:h + 1], dcols[:, jt, h:h + 1],
                    mybir.AluOpType.mult, mybir.AluOpType.max,
                )

            zt = {}  # (pair, jt) -> [P, 2, N] z tile

            def emit_z(pair, jt):
                t = wpool.tile([P, 2, N], F16, tag=f"z{pair}", bufs=NT,
                               name=f"z{pair}_{jt}")
                z_op(t[:, 0, :], 2 * pair, jt)
                z_op(t[:, 1, :], 2 * pair + 1, jt)
                zt[(pair, jt)] = t

            def mask_unit(pair, ih, jt, acc_pair):
                # one (pair, i-half, jt) unit: mask multiply + 2 PE matmuls
                sl = slice(ih * 512, (ih + 1) * 512)
                z2 = zt[(pair, jt)]
                ph = wpool.tile([P, 2, 512], F16, tag="ph", bufs=W_BUFS,
                                name=f"ph{pair}_{ih}_{jt}")
                if (pair, ih, jt) in POOL_UNITS:
                    nc.gpsimd.tensor_tensor(
                        ph[:, 0, :], z2[:, 0, sl], adjt[:, jt, sl],
                        mybir.AluOpType.mult,
                    )
                    nc.vector.tensor_tensor(
                        ph[:, 1, :], z2[:, 1, sl], adjt[:, jt, sl],
                        mybir.AluOpType.mult,
                    )
                else:
                    nc.vector.tensor_tensor(
                        ph[:], z2[:, :, sl],
                        adjt[:, jt:jt + 1, sl].to_broadcast([P, 2, 512]),
                        mybir.AluOpType.mult,
                    )
                for q in range(2):
                    nc.tensor.matmul(
                        acc_pair[q][ih][:],
                        hext[:, jt, (2 * pair + q) * DE:
                             (2 * pair + q + 1) * DE],
                        ph[:, q, :],
                        start=(jt == 0), stop=(jt == NT - 1),
                    )

            def evac_pair(pair, acc_pair):
                for q in range(2):
                    for ih in range(2):
                        nc.scalar.copy(
                            outT[2 * pair + q][:, ih * 512:(ih + 1) * 512],
                            acc_pair[q][ih][:],
                        )

            def output_pair(pair):
                ha, hb = 2 * pair, 2 * pair + 1
                po_sb = opool.tile([P, NT, 2, DE], F32, tag="posb",
                                   name=f"posb{pair}")
                for it in range(NT):
                    po = psout.tile([P, 2 * DE], F32, tag="po",
                                    name=f"po{pair}_{it}")
                    sl = slice(it * P, (it + 1) * P)
                    nc.tensor.transpose(
                        po[:, 0:DE], outT[ha][:, sl], ident[0:DE, 0:DE]
                    )
                    nc.tensor.transpose(
                        po[:, DE:2 * DE], outT[hb][:, sl], ident[0:DE, 0:DE]
                    )
                    nc.scalar.copy(
                        po_sb[:, it, :, :],
                        po[:].rearrange("p (u e) -> p u e", u=2),
                    )
                r = opool.tile([P, NT, 2], F32, tag="r", name=f"r{pair}")
                nc.vector.reciprocal(r[:], po_sb[:, :, :, D])
                nc.vector.tensor_tensor(
                    out_sb[:, :, ha * D:(hb + 1) * D]
                    .rearrange("p t (u e) -> p t u e", u=2),
                    po_sb[:, :, :, 0:D],
                    r[:, :, :, None].to_broadcast([P, NT, 2, D]),
                    mybir.AluOpType.mult,
                )

            acc01 = [
                [psagg.tile([DE, 512], F32, tag="agg", name=f"a01_{q}_{i}")
                 for i in range(2)]
                for q in range(2)
            ]
            # pair 0 (heads 0,1): half 0 while adjt cols 0:512 land first,
            # z tiles for pair 1 woven into the stream
            emit_z(0, 0)
            emit_z(0, 1)
            for jt in range(NT):
                if jt + 2 < NT:
                    emit_z(0, jt + 2)
                mask_unit(0, 0, jt, acc01)
                emit_z(1, jt)
            for jt in range(NT):
                mask_unit(0, 1, jt, acc01)
            evac_pair(0, acc01)

            # pair 1 (heads 2,3), with pair 0's output phase woven in so
            # the DVE stream never waits on the ACT/PE output chain
            acc23 = [
                [psagg.tile([DE, 512], F32, tag="agg", name=f"a23_{q}_{i}")
                 for i in range(2)]
                for q in range(2)
            ]
            for jt in range(NT):
                mask_unit(1, 0, jt, acc23)
                if jt == 2:
                    output_pair(0)
            for jt in range(NT):
                mask_unit(1, 1, jt, acc23)
            evac_pair(1, acc23)
            output_pair(1)
            nc.sync.dma_start(out_view[:, :, :], out_sb[:, :, :])

    nc.compile()
    return nc


_NC_CACHE = {}

# Test-harness knobs (not used by the grading path).
TRACE = False
LAST_RESULT = None


def _get_nc():
    if "nc" not in _NC_CACHE:
        _NC_CACHE["nc"] = build_nc()
    return _NC_CACHE["nc"]


def kernel(x, adj, W, b, a):
    global LAST_RESULT
    from concourse.bass_utils import run_bass_kernel_spmd

    nc = _get_nc()
    x = np.asarray(x, dtype=np.float32)
    adj = np.asarray(adj, dtype=np.int32)
    W = np.ascontiguousarray(np.asarray(W, dtype=np.float32))
    b = np.ascontiguousarray(np.asarray(b, dtype=np.float32))
    a = np.ascontiguousarray(np.asarray(a, dtype=np.float32))

    in_maps = [
        {
            "x": np.ascontiguousarray(x[c]),
            "adj": np.ascontiguousarray(adj[c]),
            "W": W,
            "b": b,
            "a": a,
        }
        for c in range(NCORES)
    ]
    res = run_bass_kernel_spmd(
        nc, in_maps, core_ids=list(range(NCORES)), trace=TRACE
    )
    LAST_RESULT = res
    out = np.stack([res.results[c]["out"] for c in range(NCORES)], axis=0)
    return out.astype(np.float32)


if __name__ == "__main__":
    nc = build_nc()
    print("built OK")
